# revision 6
# baseline (speedup 1.0000x reference)
"""Trainium2 Bass kernel for a 2-layer ResGatedGraphConv encoder.

Strategy (edge-parallel over 8 NeuronCores):
  - Nodes are permuted by degree rank and dealt round-robin to the 8 cores, so
    each core owns NPC nodes arranged in TPC tiles of 128 dst nodes whose
    degrees are nearly uniform within a tile.
  - Each edge lives on the core/tile/partition of its dst node.  Tiles are
    packed into GROUPS with a uniform padded degree dh_g (d_hat is sorted, so
    padding is small); per group ONE batched indirect DMA gathers every edge's
    [q|v] table row into a token-major [128, T*dh_g, 128] SBUF tile.
  - Per layer a packed [q|v] node table [NT, 128] (bf16) is built on device
    with PE matmuls and written to DRAM.  k is only needed per dst node: it is
    computed per tile [128, 64] and broadcast along the degree axis.
  - Messages: one group-wide add (k broadcast), sigmoid, multiply; a fold tree
    reduces the degree axis; per-tile PE transpose + fused linear.
  - h1 is exchanged between cores with an AllGather (bf16) so layer 2 can
    build its node table from the full hidden state.
  - Bias algebra is folded on the host: (agg + x@Ws + b) @ Wl + bl
    = agg@Wl + x@(Ws@Wl) + (b@Wl + bl).

kernel(**inputs) takes the full (unsharded) inputs and returns the full
output; all sharding happens inside.
"""

import sys
import numpy as np

for _p in ("/opt/trn_rl_repo", "/opt/pypackages"):
    if _p not in sys.path:
        sys.path.append(_p)

N = 100000
E = 1600000
H = 64
NCORES = 8
GCOLS = 96       # max padded columns (T * dh_g) per gather group
GTILES = 8       # max tiles per group (ksb PSUM bank: T*64 <= 512)


class Cfg:
    def __init__(self, n, tpc):
        self.n = n
        self.tpc = tpc                      # dst tiles per core
        self.npc = tpc * 128                # nodes per core
        self.npad = NCORES * self.npc       # padded node count
        self.nt = self.npad + 128           # table rows (incl. dummy block)
        self.dummy = self.npad              # dummy (all-zero) table row
        assert self.npad >= n
        assert self.npad % 512 == 0


FULL_CFG = Cfg(N, 98)


def make_groups(d_hat):
    """Pack consecutive tiles into groups with uniform padded degree.

    d_hat is ascending, so padding to the group max is cheap.
    Returns list of (t0, ntiles, dh_g, col0) and total padded columns.
    """
    groups = []
    col0 = 0
    t0 = 0
    tpc = len(d_hat)
    while t0 < tpc:
        nt_g = 1
        dh_g = d_hat[t0]
        while (t0 + nt_g < tpc and nt_g < GTILES
               and (nt_g + 1) * d_hat[t0 + nt_g] <= GCOLS):
            nt_g += 1
            dh_g = d_hat[t0 + nt_g - 1]
        groups.append((t0, nt_g, dh_g, col0))
        col0 += nt_g * dh_g
        t0 += nt_g
    return groups, col0


def host_prep(x, edge_index, cfg):
    """Permute nodes / build per-core gather schedules on the host."""
    n = cfg.n
    src = np.asarray(edge_index[0]).astype(np.int64)
    dst = np.asarray(edge_index[1]).astype(np.int64)
    deg = np.bincount(dst, minlength=n)

    # degree-rank round-robin: rank r -> core r%8, slot r//8
    rank_order = np.argsort(deg, kind="stable")  # node ids in degree order
    r = np.arange(n)
    node_core = np.empty(n, np.int64)
    node_slot = np.empty(n, np.int64)
    node_core[rank_order] = r % NCORES
    node_slot[rank_order] = r // NCORES
    tau = node_core * cfg.npc + node_slot      # table id of each node

    # per-edge position within its dst node's list
    order = np.argsort(dst, kind="stable")
    d_sorted = dst[order]
    first = np.searchsorted(d_sorted, np.arange(n))
    k_within = np.arange(len(dst)) - first[d_sorted]
    k_e = np.empty(len(dst), np.int64)
    k_e[order] = k_within

    e_core = node_core[dst]
    e_slot = node_slot[dst]
    e_tile = e_slot // 128
    e_part = e_slot % 128

    # per-tile max degree (shared across cores so programs are identical)
    deg_cs = np.zeros((NCORES, cfg.npc), np.int64)
    deg_cs[node_core, node_slot] = deg
    d_hat = deg_cs.reshape(NCORES, cfg.tpc, 128).max(axis=(0, 2))
    d_hat = np.maximum(d_hat, 1).astype(np.int64)

    groups, nblk = make_groups([int(v) for v in d_hat])
    # column offset of each tile inside the group-padded gidx layout
    tile_col = np.zeros(cfg.tpc, np.int64)
    for (t0, nt_g, dh_g, col0) in groups:
        for i in range(nt_g):
            tile_col[t0 + i] = col0 + i * dh_g

    tau_src = tau[src]
    gidx = np.full((NCORES, 128, nblk), cfg.dummy, np.int32)
    col = tile_col[e_tile] + k_e
    gidx[e_core, e_part, col] = tau_src.astype(np.int32)

    # permuted feature table input, feature-major, zero padded, bf16
    bf16 = _bf16_dtype()
    xT_full = np.zeros((H, cfg.nt), np.float32)
    xT_full[:, tau] = np.asarray(x, np.float32).T
    xT_full = xT_full.astype(bf16)

    return dict(
        gidx=gidx,
        d_hat=[int(v) for v in d_hat],
        groups=groups,
        nblk=nblk,
        tau=tau,
        xT_full=xT_full,
    )


def _bf16_dtype():
    import concourse.mybir as mybir
    return mybir.dt.np(mybir.dt.bfloat16)


def build_program(cfg, d_hat, groups, nblk, gather_mode="group"):
    import concourse.bass as bass
    import concourse.bacc as bacc
    import concourse.mybir as mybir
    import concourse.tile as tile
    from concourse.masks import make_identity

    f32 = mybir.dt.float32
    bf16 = mybir.dt.bfloat16
    tpc, npc, nt = cfg.tpc, cfg.npc, cfg.nt
    npad = cfg.npad

    nc = bacc.Bacc("TRN2", target_bir_lowering=False, debug=False,
                   num_devices=NCORES)

    # ---- I/O ----
    xT_full = nc.dram_tensor("xT_full", [H, nt], bf16, kind="ExternalInput")
    xT_own = nc.dram_tensor("xT_own", [H, npc], bf16, kind="ExternalInput")
    gidx = nc.dram_tensor("gidx", [128, nblk], mybir.dt.int32,
                          kind="ExternalInput")
    wnames = {}
    for l in (1, 2):
        for w in ("Wqv", "Wk", "Wsl", "Wl"):
            shape = [H, 128] if w == "Wqv" else [H, H]
            wnames[f"{w}{l}"] = nc.dram_tensor(f"{w}{l}", shape, bf16,
                                               kind="ExternalInput")
        wnames[f"blp{l}"] = nc.dram_tensor(f"blp{l}", [H, 1], f32,
                                           kind="ExternalInput")
    out_shard = nc.dram_tensor("out_shard", [H, npc], f32,
                               kind="ExternalOutput")

    # ---- internal DRAM ----
    qv_t = [nc.dram_tensor(f"qv{l}_t", [nt, 128], bf16) for l in (1, 2)]
    h1_shard = nc.dram_tensor("h1_shard", [H, npc], bf16)
    h1_gath = nc.dram_tensor("h1_gath", [NCORES * H, npc], bf16,
                             addr_space="Shared")

    with tile.TileContext(nc) as tc:
        cp = tc.alloc_tile_pool(name="const", bufs=1)

        identity = cp.tile([128, 128], f32)
        make_identity(nc, identity[:])

        gidx_sb = cp.tile([128, nblk], mybir.dt.int32)
        nc.sync.dma_start(out=gidx_sb[:], in_=gidx.ap()[:])

        wt = {}
        for l in (1, 2):
            for w in ("Wqv", "Wk", "Wsl", "Wl"):
                shape = [H, 128] if w == "Wqv" else [H, H]
                wt[f"{w}{l}"] = cp.tile(shape, bf16, name=f"{w}{l}",
                                        tag=f"{w}{l}")
                nc.sync.dma_start(out=wt[f"{w}{l}"][:],
                                  in_=wnames[f"{w}{l}"].ap()[:])
            wt[f"blp{l}"] = cp.tile([H, 1], f32, name=f"blp{l}", tag=f"blp{l}")
            nc.sync.dma_start(out=wt[f"blp{l}"][:],
                              in_=wnames[f"blp{l}"].ap()[:])

        # dummy (all-zero) table rows [npad:nt) for both layers
        zrow = cp.tile([128, 128], bf16)
        nc.vector.memset(zrow[:], 0.0)
        for l in (0, 1):
            nc.sync.dma_start(out=qv_t[l].ap()[npad:nt, :], in_=zrow[:])

        # ---------------- phase A: build [q|v] table ----------------
        CH = 4  # table tiles per chunk (one PSUM bank: 4*128 = 512 fp32)

        def phase_a(layer):
            table = qv_t[layer - 1]
            wqv = wt[f"Wqv{layer}"]
            ntiles = npad // 128
            with tc.tile_pool(name=f"pa{layer}", bufs=3) as pa, \
                 tc.tile_pool(name=f"pap{layer}", bufs=2, space="PSUM") as pap:
                for j0 in range(0, ntiles, CH):
                    src_t = pa.tile([H, CH * 128], bf16, tag="src")
                    if layer == 1:
                        nc.sync.dma_start(
                            out=src_t[:],
                            in_=xT_full.ap()[:, j0 * 128:(j0 + CH) * 128])
                    else:
                        # source h1_gath with per-core reslicing; a chunk may
                        # cross a core boundary
                        seg0 = 0
                        while seg0 < CH * 128:
                            g = j0 * 128 + seg0
                            c = g // npc
                            n0 = g % npc
                            seglen = min(CH * 128 - seg0, npc - n0)
                            nc.sync.dma_start(
                                out=src_t[:, seg0:seg0 + seglen],
                                in_=h1_gath.ap()[c * H:(c + 1) * H,
                                                 n0:n0 + seglen])
                            seg0 += seglen
                    ps = pap.tile([128, CH * 128], f32, tag="ps", space="PSUM")
                    for i in range(CH):
                        nc.tensor.matmul(
                            ps[:, i * 128:(i + 1) * 128],
                            lhsT=src_t[:, i * 128:(i + 1) * 128],
                            rhs=wqv[:], start=True, stop=True)
                    st = pa.tile([128, CH * 128], bf16, tag="st")
                    nc.scalar.activation(st[:], ps[:],
                                         mybir.ActivationFunctionType.Copy)
                    out_ap = table.ap()[j0 * 128:(j0 + CH) * 128, :]
                    out_ap = out_ap.rearrange("(c p) e -> p c e", p=128)
                    in_ap = st[:].rearrange("p (c e) -> p c e", e=128)
                    nc.sync.dma_start(out=out_ap, in_=in_ap)

        # ---------------- phase B: gated conv + fused linear ----------------
        def conv_layer(layer):
            table = qv_t[layer - 1]
            hsrc_dram = xT_own if layer == 1 else h1_shard
            wk, wsl, wl = wt[f"Wk{layer}"], wt[f"Wsl{layer}"], wt[f"Wl{layer}"]
            blp = wt[f"blp{layer}"]
            odram = h1_shard if layer == 1 else out_shard
            odt = bf16 if layer == 1 else f32
            with tc.tile_pool(name=f"pb{layer}", bufs=2) as pb, \
                 tc.tile_pool(name=f"pbp{layer}", bufs=2, space="PSUM") as pbp:
                for (t0, T, dh, col0) in groups:
                    C = T * dh
                    # per-group loads
                    hot = pb.tile([H, GTILES * 128], bf16, tag="hot")
                    nc.sync.dma_start(
                        out=hot[:, 0:T * 128],
                        in_=hsrc_dram.ap()[:, t0 * 128:(t0 + T) * 128])
                    qvg_f = pb.tile([128, GCOLS * 128], bf16, tag="qvg")
                    if gather_mode == "group":
                        nc.gpsimd.indirect_dma_start(
                            out=qvg_f[:, 0:C * 128].rearrange(
                                "p (c e) -> p c e", e=128),
                            out_offset=None,
                            in_=table.ap()[:, :],
                            in_offset=bass.IndirectOffsetOnAxis(
                                ap=gidx_sb[:, col0:col0 + C], axis=0),
                        )
                    elif gather_mode == "tile":
                        for i in range(T):
                            nc.gpsimd.indirect_dma_start(
                                out=qvg_f[:, i * dh * 128:(i + 1) * dh * 128]
                                .rearrange("p (c e) -> p c e", e=128),
                                out_offset=None,
                                in_=table.ap()[:, :],
                                in_offset=bass.IndirectOffsetOnAxis(
                                    ap=gidx_sb[:, col0 + i * dh:
                                               col0 + (i + 1) * dh], axis=0),
                            )
                    else:  # per-column (baseline mechanism)
                        for j in range(C):
                            nc.gpsimd.indirect_dma_start(
                                out=qvg_f[:, j * 128:(j + 1) * 128],
                                out_offset=None,
                                in_=table.ap()[:, :],
                                in_offset=bass.IndirectOffsetOnAxis(
                                    ap=gidx_sb[:, col0 + j:col0 + j + 1],
                                    axis=0),
                            )
                    qvg = qvg_f[:, 0:C * 128].rearrange(
                        "p (t k e) -> p t k e", k=dh, e=128)

                    # k for all tiles of the group: PSUM bank [128, T*64]
                    kps = pbp.tile([128, GTILES * H], f32, tag="kps",
                                   space="PSUM")
                    for i in range(T):
                        nc.tensor.matmul(
                            kps[:, i * H:(i + 1) * H],
                            lhsT=hot[:, i * 128:(i + 1) * 128],
                            rhs=wk[:], start=True, stop=True)
                    ksb = pb.tile([128, GTILES * H], bf16, tag="ksb")
                    nc.scalar.activation(ksb[:, 0:T * H], kps[:, 0:T * H],
                                         mybir.ActivationFunctionType.Copy)

                    # sigarg = q + k[dst]  (k broadcast along degree axis)
                    sigarg = pb.tile([128, GCOLS * H], bf16, tag="sigarg")
                    sa = sigarg[:, 0:C * H].rearrange(
                        "p (t k h) -> p t k h", k=dh, h=H)
                    kb = ksb[:, 0:T * H].rearrange(
                        "p (t o h) -> p t o h", o=1, h=H)
                    kb = bass.AP(kb.tensor, kb.offset,
                                 [kb.ap[0], kb.ap[1], [0, dh], kb.ap[3]])
                    nc.vector.tensor_tensor(
                        out=sa, in0=qvg[:, :, :, 0:H], in1=kb,
                        op=mybir.AluOpType.add)
                    # sig = sigmoid(sigarg), one op for the whole group
                    sig = pb.tile([128, GCOLS * H], bf16, tag="sig")
                    nc.scalar.activation(
                        sig[:, 0:C * H], sigarg[:, 0:C * H],
                        mybir.ActivationFunctionType.Sigmoid)
                    # msg = sig * v, one op for the whole group
                    msg = pb.tile([128, GCOLS * H], bf16, tag="msg")
                    nc.vector.tensor_tensor(
                        out=msg[:, 0:C * H].rearrange(
                            "p (t k h) -> p t k h", k=dh, h=H),
                        in0=sig[:, 0:C * H].rearrange(
                            "p (t k h) -> p t k h", k=dh, h=H),
                        in1=qvg[:, :, :, H:128],
                        op=mybir.AluOpType.mult)

                    # fold tree over the degree axis (all tiles at once)
                    mv = msg[:, 0:C * H].rearrange(
                        "p (t k h) -> p t k h", k=dh, h=H)
                    cur = dh
                    while cur > 2:
                        k2 = cur // 2
                        nc.vector.tensor_tensor(
                            out=mv[:, :, 0:k2, :],
                            in0=mv[:, :, 0:k2, :],
                            in1=mv[:, :, cur - k2:cur, :],
                            op=mybir.AluOpType.add)
                        cur -= k2
                    agg = pb.tile([128, GTILES * H], f32, tag="agg")
                    av = agg[:, 0:T * H].rearrange("p (t h) -> p t h", h=H)
                    if cur == 2:
                        nc.vector.tensor_tensor(
                            out=av, in0=mv[:, :, 0, :], in1=mv[:, :, 1, :],
                            op=mybir.AluOpType.add)
                    else:
                        nc.vector.tensor_copy(av, mv[:, :, 0, :])

                    # transpose agg tiles to feature-major, 4 tiles per bank
                    ob = pb.tile([H, GTILES * 128], odt, tag="ob")
                    for b0 in range(0, T, 4):
                        nb = min(4, T - b0)
                        tps = pbp.tile([H, 4 * 128], f32, tag="tps",
                                       space="PSUM")
                        for i in range(nb):
                            nc.tensor.transpose(
                                out=tps[:, i * 128:(i + 1) * 128],
                                in_=agg[:, (b0 + i) * H:(b0 + i + 1) * H],
                                identity=identity[:])
                        aggT = pb.tile([H, 4 * 128], bf16, tag="aggT")
                        nc.scalar.activation(
                            aggT[:, 0:nb * 128], tps[:, 0:nb * 128],
                            mybir.ActivationFunctionType.Copy)
                        # fused linear: agg@Wl + x@(Ws@Wl)
                        lps = pbp.tile([H, 4 * 128], f32, tag="lps",
                                       space="PSUM")
                        nc.tensor.matmul(lps[:, 0:nb * 128], lhsT=wl[:],
                                         rhs=aggT[:, 0:nb * 128],
                                         start=True, stop=False)
                        nc.tensor.matmul(
                            lps[:, 0:nb * 128], lhsT=wsl[:],
                            rhs=hot[:, b0 * 128:(b0 + nb) * 128],
                            start=False, stop=True)
                        nc.scalar.activation(
                            ob[:, b0 * 128:(b0 + nb) * 128],
                            lps[:, 0:nb * 128],
                            mybir.ActivationFunctionType.Relu,
                            bias=blp[:])
                    nc.sync.dma_start(
                        out=odram.ap()[:, t0 * 128:(t0 + T) * 128],
                        in_=ob[:, 0:T * 128])

        phase_a(1)
        tc.strict_bb_all_engine_barrier()
        conv_layer(1)

        # exchange h1 across cores
        nc.gpsimd.collective_compute(
            "AllGather",
            mybir.AluOpType.bypass,
            replica_groups=[list(range(NCORES))],
            ins=[h1_shard.ap()[:, :]],
            outs=[h1_gath.ap()[:, :]],
        )

        phase_a(2)
        tc.strict_bb_all_engine_barrier()
        conv_layer(2)

        cp.release()

    nc.compile()
    return nc


def _pack_inputs(prep, inputs, cfg):
    """Build the 8 per-core input maps."""
    bf16 = _bf16_dtype()
    xT_full = prep["xT_full"]
    base = {"xT_full": xT_full}
    for l, (wq, wv, wk, ws, b, wl, bl) in {
        1: ("Wq1", "Wv1", "Wk1", "Ws1", "b1", "Wl1", "bl1"),
        2: ("Wq2", "Wv2", "Wk2", "Ws2", "b2", "Wl2", "bl2"),
    }.items():
        Wq = np.asarray(inputs[wq], np.float32)
        Wv = np.asarray(inputs[wv], np.float32)
        Wk = np.asarray(inputs[wk], np.float32)
        Ws = np.asarray(inputs[ws], np.float32)
        Wl = np.asarray(inputs[wl], np.float32)
        b = np.asarray(inputs[b], np.float32)
        bl = np.asarray(inputs[bl], np.float32)
        base[f"Wqv{l}"] = np.ascontiguousarray(
            np.concatenate([Wq, Wv], axis=1)).astype(bf16)
        base[f"Wk{l}"] = np.ascontiguousarray(Wk).astype(bf16)
        base[f"Wsl{l}"] = np.ascontiguousarray(Ws @ Wl).astype(bf16)
        base[f"Wl{l}"] = np.ascontiguousarray(Wl).astype(bf16)
        base[f"blp{l}"] = np.ascontiguousarray(
            (b @ Wl + bl).reshape(H, 1).astype(np.float32))

    in_maps = []
    for c in range(NCORES):
        m = dict(base)
        m["xT_own"] = np.ascontiguousarray(
            xT_full[:, c * cfg.npc:(c + 1) * cfg.npc])
        m["gidx"] = np.ascontiguousarray(prep["gidx"][c])
        in_maps.append(m)
    return in_maps


def run(inputs, cfg=FULL_CFG, sim=False, trace=False):
    from concourse import bass_utils

    x = np.asarray(inputs["x"], np.float32)
    prep = host_prep(x, inputs["edge_index"], cfg)
    import os
    gather_mode = os.environ.get("GATHER_MODE", "group")
    nc = build_program(cfg, prep["d_hat"], prep["groups"], prep["nblk"],
                       gather_mode=gather_mode)
    in_maps = _pack_inputs(prep, inputs, cfg)

    if sim:
        from concourse.bass_interp import MultiCoreSim
        ms = MultiCoreSim(nc, num_cores=NCORES, trace=False)
        for c in range(NCORES):
            for name, arr in in_maps[c].items():
                ms.cores[c].tensor(name)[:] = arr
        ms.simulate(check_with_hw=False)
        shards = [np.array(ms.cores[c].tensor("out_shard")) for c in
                  range(NCORES)]
        res = None
    else:
        if trace:
            try:
                sys.path.insert(0, "/root/problem")
                import ntff_hook  # noqa: F401
            except Exception:
                trace = False
        res = bass_utils.run_bass_kernel_spmd(
            nc, in_maps, core_ids=list(range(NCORES)), trace=trace)
        shards = [res.results[c]["out_shard"] for c in range(NCORES)]

    full_T = np.concatenate(shards, axis=1)   # [H, npad] in permuted order
    out = np.ascontiguousarray(full_T[:, prep["tau"]].T.astype(np.float32))
    return out, res


def kernel(**inputs):
    out, _ = run(inputs, FULL_CFG, sim=False, trace=False)
    return out.astype(np.float32)
